# revision 12
# baseline (speedup 1.0000x reference)
"""Mixture causal self-attention (NAS weight-entanglement supernet) on 8 trn2 cores.

Math (validated vs reference):
  Wmix = W * s[max(row%C, col)] with staircase s from softmax(alpha_embed).
  qkv = x @ Wmix_attn.T ; y = sum over 9 (h,e) combos of w_he * Attn_he(q,k,v);
  out = y @ Wmix_proj.T.
Key reduction: combos with equal head dim d=e/h produce IDENTICAL per-slice
attention outputs, so the 9 combos collapse to 60 unique (d, slice) units,
accumulated with staircase weights:
  d=16: 16 slices, d=32: 16, d=64: 16, d=128: 8, d=256: 4.
Sharding: core pair (2b, 2b+1) owns batch b; even cores take the first half of
each d-group's slices (features [0:512]), odd cores the second half. Identical
SPMD program; per-core weight/coef data differ. Host sums the pair partials.
"""

import numpy as np
import ml_dtypes

C_MAX = 1024
T = 1024
B = 4
N_CORES = 8

# local slice list (per core), in processing order: copies into y before adds
# (d64 tiles y cols [384:896] disjointly -> copy; d128/d256 overlap -> add)
SLICES = (
    [(64, j) for j in range(8)]
    + [(128, j) for j in range(4)]
    + [(256, j) for j in range(2)]
    + [(32, j) for j in range(8)]
    + [(16, j) for j in range(8)]
)
N_LOCAL = {16: 8, 32: 8, 64: 8, 128: 4, 256: 2}
# packed q/k row space [0:896]: small = d16 feats, mid = d32 feats, big = rest
QOFF = {16: 0, 32: 128}  # d64/128/256 live in big block [384:896] via tQ128
# Vx (V with interleaved ones col per slice) group offsets
VXOFF = {}
_off = 0
for _d in (64, 128, 256, 32, 16):
    VXOFF[_d] = _off
    _off += (_d + 1) * N_LOCAL[_d]
VX_W = _off  # 1950
# y packed col offsets (no ones cols)
YOFF = {16: 0, 32: 128, 64: 384, 128: 384, 256: 384}
PACK_W = 896

_BUILT = {}


def _softmax1d(v):
    v = v - v.max()
    e = np.exp(v)
    return e / e.sum()


def _combo_weights(alpha_embed, alpha_heads):
    ae = _softmax1d(np.asarray(alpha_embed, np.float64))
    ah = _softmax1d(np.asarray(alpha_heads, np.float64))
    w = {}
    for hi, h in enumerate((4, 8, 16)):
        for ei, e in enumerate((256, 512, 1024)):
            w[(h, e)] = float(ah[hi] * ae[ei])
    return ae, w


def _stair_coef(d, gj, w):
    # weight of global slice (d, gj) = sum of w[(h, h*d)] over combos with h > gj
    c = 0.0
    for h in (4, 8, 16):
        e = h * d
        if e in (256, 512, 1024) and gj < h:
            c += w[(h, e)]
    return c


def _build_bass():
    import concourse.bass as bass
    from concourse import bacc
    import concourse.mybir as mybir
    import concourse.tile as tile
    from concourse.masks import make_identity

    bf16 = mybir.dt.bfloat16
    f32 = mybir.dt.float32
    AF = mybir.ActivationFunctionType

    nc = bacc.Bacc()
    xT = nc.dram_tensor("xT", [128, 8, T], bf16, kind="ExternalInput")
    wqk = nc.dram_tensor("wqk", [128, 14, 8, 128], bf16, kind="ExternalInput")
    wv = nc.dram_tensor("wv", [128, 8, PACK_W], bf16, kind="ExternalInput")
    wp = nc.dram_tensor("wp", [128, 7, C_MAX], bf16, kind="ExternalInput")
    coefs = nc.dram_tensor("coefs", [128, len(SLICES)], f32, kind="ExternalInput")
    tri = nc.dram_tensor("tri", [128, 128], bf16, kind="ExternalInput")
    out = nc.dram_tensor("out", [T, C_MAX], f32, kind="ExternalOutput")

    # E strips packed into 5 groups: (kb0), (kb1,kb7), (kb2,kb6), (kb3,kb5), (kb4)
    # one PSUM tile + one exp per group
    KB_GROUPS = [(0,), (1, 7), (2, 6), (3, 5), (4,)]
    EOFF = {}
    _e = 0
    for grp in KB_GROUPS:
        for kb in grp:
            EOFF[kb] = _e
            _e += 1024 - 128 * kb
    E_W = _e  # 4608
    # within-psum-tile col offset of each kb (strips packed in group order)
    PSOFF = {}
    for grp in KB_GROUPS:
        _o = 0
        for kb in grp:
            PSOFF[kb] = _o
            _o += 1024 - 128 * kb

    with tile.TileContext(nc) as tc:
        with tc.tile_pool(name="persist", bufs=1) as P:
            ttri = P.tile([128, 128], bf16, name="ttri")
            nc.sync.dma_start(out=ttri, in_=tri[:, :])
            tcoef = P.tile([128, len(SLICES)], f32, name="tcoef")
            nc.sync.dma_start(out=tcoef, in_=coefs[:, :])
            tident = P.tile([128, 128], f32, name="tident")
            make_identity(nc, tident)

            # persistent activation tensors
            tQ16 = P.tile([16, 8, 1024], bf16, name="tQ16")
            tK16 = P.tile([16, 8, 1024], bf16, name="tK16")
            tQ32 = P.tile([32, 8, 1024], bf16, name="tQ32")
            tK32 = P.tile([32, 8, 1024], bf16, name="tK32")
            tQ128 = P.tile([128, 4, 1024], bf16, name="tQ128")
            tK128 = P.tile([128, 4, 1024], bf16, name="tK128")
            tVx = P.tile([128, 8, VX_W], bf16, name="tVx")
            tY = P.tile([128, 8, PACK_W], f32, name="tY")

            GQ = {16: tQ16, 32: tQ32}
            GK = {16: tK16, 32: tK32}

            # ---------------- phase 1: qkv projection ----------------
            with tc.tile_pool(name="p1sb", bufs=3) as S1, \
                 tc.tile_pool(name="p1w", bufs=4) as WP, \
                 tc.tile_pool(name="p1x", bufs=1) as XP, \
                 tc.tile_pool(name="pqk", bufs=2, space="PSUM") as PQK, \
                 tc.tile_pool(name="pv", bufs=2, space="PSUM") as PV:
                tX = XP.tile([128, 8, 1024], bf16, name="tX")
                nc.sync.dma_start(out=tX, in_=xT[:, :, :])
                twv = XP.tile([128, 8, PACK_W], bf16, name="twv")
                nc.sync.dma_start(out=twv, in_=wv[:, :, :])

                # Q^T / K^T packed rows, o-chunks of 128
                for oc in range(14):
                    pqk = PQK.tile([128, 1024], f32, name="pqk")
                    wt = WP.tile([128, 8, 128], bf16, name="wt")
                    nc.sync.dma_start(out=wt, in_=wqk[:, oc, :, :])
                    for cc in range(8):
                        nc.tensor.matmul(pqk[:, 0:512], wt[:, cc, :], tX[:, cc, 0:512],
                                         start=(cc == 0), stop=(cc == 7))
                        nc.tensor.matmul(pqk[:, 512:1024], wt[:, cc, :], tX[:, cc, 512:1024],
                                         start=(cc == 0), stop=(cc == 7))
                    isq, sub = divmod(oc, 7)
                    gdst = (GQ if isq == 0 else GK)
                    if sub == 0:  # small block: 8 d16 slices of 16 rows
                        stg = S1.tile([128, 1024], bf16, name="stg")
                        nc.vector.tensor_copy(stg, pqk)
                        for s in range(8):
                            nc.sync.dma_start(out=gdst[16][:, s, :], in_=stg[16 * s:16 * (s + 1), :])
                    elif sub in (1, 2):  # mid block: 4 d32 slices per chunk
                        stg = S1.tile([128, 1024], bf16, name="stg")
                        nc.vector.tensor_copy(stg, pqk)
                        for s in range(4):
                            nc.sync.dma_start(out=gdst[32][:, 4 * (sub - 1) + s, :],
                                              in_=stg[32 * s:32 * (s + 1), :])
                    else:  # big block: direct
                        dst = (tQ128 if isq == 0 else tK128)
                        nc.vector.tensor_copy(dst[:, sub - 3, :], pqk)

                # V (normal layout) -> interleaved Vx with ones columns
                for tc2 in range(8):
                    pv = PV.tile([128, 896], f32, name="pv")
                    for cc in range(8):
                        nc.tensor.matmul(pv[:, 0:512], tX[:, cc, 128 * tc2:128 * (tc2 + 1)],
                                         twv[:, cc, 0:512], start=(cc == 0), stop=(cc == 7))
                        nc.tensor.matmul(pv[:, 512:896], tX[:, cc, 128 * tc2:128 * (tc2 + 1)],
                                         twv[:, cc, 512:896], start=(cc == 0), stop=(cc == 7))
                    for d in (64, 128, 256, 32, 16):
                        n = N_LOCAL[d]
                        # copy [128, n, d] from packed V cols -> Vx strided (d+1)
                        voff = YOFF[d]
                        nc.vector.tensor_copy(
                            tVx[:, tc2, VXOFF[d]:VXOFF[d] + (d + 1) * n]
                            .rearrange("p (s e) -> p s e", e=d + 1)[:, :, 0:d],
                            pv[:, voff:voff + d * n].rearrange("p (s e) -> p s e", e=d))
                # ones columns
                for d in (64, 128, 256, 32, 16):
                    n = N_LOCAL[d]
                    nc.vector.memset(
                        tVx[:, :, VXOFF[d]:VXOFF[d] + (d + 1) * n]
                        .rearrange("p t (s e) -> p t s e", e=d + 1)[:, :, :, d:d + 1], 1.0)

            # ---------------- phase 2: attention slices ----------------
            # software pipeline: scores+exp for slice pair p overlap E@V of
            # pair p-1 (PE works E@V while ACT runs exp of the next pair)
            with tc.tile_pool(name="p2e", bufs=5) as EP, \
                 tc.tile_pool(name="p2v", bufs=6) as DVP, \
                 tc.tile_pool(name="pss", bufs=3, space="PSUM") as PSS, \
                 tc.tile_pool(name="pso", bufs=2, space="PSUM") as PSO:

                def emit_scores_group(d, lj, tE, grp, scale):
                    ps = PSS.tile([128, 1024], f32, name="ps")
                    gw = 0
                    for kb in grp:
                        w = 1024 - 128 * kb
                        base = PSOFF[kb]
                        # matmul out chunks within one psum bank each
                        cuts = [base]
                        for b in (512, 1024):
                            if base < b < base + w:
                                cuts.append(b)
                        cuts.append(base + w)
                        if d <= 64:
                            if d <= 32:
                                lk = GK[d][:, lj, :]
                                lq = GQ[d][:, lj, :]
                            else:
                                p0 = 64 * (lj % 2)
                                lk = tK128[p0:p0 + 64, lj // 2, :]
                                lq = tQ128[p0:p0 + 64, lj // 2, :]
                            for a, b in zip(cuts[:-1], cuts[1:]):
                                qo = 128 * kb + (a - base)
                                nc.tensor.matmul(
                                    ps[:, a:b], lk[:, 128 * kb:128 * (kb + 1)],
                                    lq[:, qo:qo + (b - a)], start=True, stop=True)
                        elif d == 128:
                            for a, b in zip(cuts[:-1], cuts[1:]):
                                qo = 128 * kb + (a - base)
                                nc.tensor.matmul(
                                    ps[:, a:b], tK128[:, lj, 128 * kb:128 * (kb + 1)],
                                    tQ128[:, lj, qo:qo + (b - a)], start=True, stop=True)
                        else:
                            for a, b in zip(cuts[:-1], cuts[1:]):
                                qo = 128 * kb + (a - base)
                                for h2 in range(2):
                                    nc.tensor.matmul(
                                        ps[:, a:b], tK128[:, 2 * lj + h2, 128 * kb:128 * (kb + 1)],
                                        tQ128[:, 2 * lj + h2, qo:qo + (b - a)],
                                        start=(h2 == 0), stop=(h2 == 1))
                        gw += w
                    # one exp for the whole packed group
                    kb0 = grp[0]
                    nc.scalar.activation(tE[:, EOFF[kb0]:EOFF[kb0] + gw], ps[:, 0:gw],
                                         AF.Exp, scale=scale)
                    for kb in grp:
                        nc.gpsimd.affine_select(
                            out=tE[:, EOFF[kb]:EOFF[kb] + 128],
                            in_=tE[:, EOFF[kb]:EOFF[kb] + 128],
                            compare_op=mybir.AluOpType.is_ge,
                            fill=0.0, base=0, pattern=[[1, 128]], channel_multiplier=-1)

                def emit_eav(si, d, lj, tE):
                    for qi in range(8):
                        po = PSO.tile([128, 257], f32, name="po")
                        for kb in range(qi + 1):
                            nc.tensor.matmul(
                                po[:, 0:d + 1],
                                tE[:, EOFF[kb] + 128 * (qi - kb):EOFF[kb] + 128 * (qi - kb) + 128],
                                tVx[:, kb, VXOFF[d] + (d + 1) * lj:VXOFF[d] + (d + 1) * (lj + 1)],
                                start=(kb == 0), stop=(kb == qi))
                        tdin = DVP.tile([128, 1], f32, name="tdin")
                        nc.vector.reciprocal(tdin, po[:, d:d + 1])
                        ycol = YOFF[d] + d * lj
                        if d in (128, 256):  # accumulate
                            ttmp = DVP.tile([128, 256], f32, name="ttmp")
                            nc.vector.tensor_scalar(
                                out=ttmp[:, 0:d], in0=po[:, 0:d], scalar1=tdin,
                                scalar2=tcoef[:, si:si + 1],
                                op0=mybir.AluOpType.mult, op1=mybir.AluOpType.mult)
                            nc.vector.tensor_add(tY[:, qi, ycol:ycol + d],
                                                 tY[:, qi, ycol:ycol + d], ttmp[:, 0:d])
                        else:  # first (only) writer of these y cols
                            nc.vector.tensor_scalar(
                                out=tY[:, qi, ycol:ycol + d], in0=po[:, 0:d], scalar1=tdin,
                                scalar2=tcoef[:, si:si + 1],
                                op0=mybir.AluOpType.mult, op1=mybir.AluOpType.mult)

                prev = None
                for pi in range(0, len(SLICES), 2):
                    cur = []
                    for si in (pi, pi + 1):
                        d, lj = SLICES[si]
                        tE = EP.tile([128, E_W], bf16, name="tE")
                        cur.append((si, d, lj, tE))
                    for grp in KB_GROUPS:
                        for (si, d, lj, tE) in cur:
                            emit_scores_group(d, lj, tE, grp, float(1.0 / np.sqrt(d)))
                    if prev is not None:
                        for (si, d, lj, tE) in prev:
                            emit_eav(si, d, lj, tE)
                    prev = cur
                for (si, d, lj, tE) in prev:
                    emit_eav(si, d, lj, tE)

            # ---------------- phase 3: transpose y, c_proj ----------------
            with tc.tile_pool(name="p3sb", bufs=2) as S3, \
                 tc.tile_pool(name="p3yt", bufs=1) as YTP, \
                 tc.tile_pool(name="p3w", bufs=1) as WP3, \
                 tc.tile_pool(name="pst", bufs=2, space="PSUM") as PST, \
                 tc.tile_pool(name="psc", bufs=2, space="PSUM") as PSC:
                twp = WP3.tile([128, 7, 1024], bf16, name="twp")
                nc.sync.dma_start(out=twp, in_=wp[:, :, :])
                tYT = YTP.tile([128, 7, 1024], bf16, name="tYT")
                for cc in range(7):
                    for tc2 in range(8):
                        pt = PST.tile([128, 128], f32, name="pt")
                        nc.tensor.transpose(pt, tY[:, tc2, 128 * cc:128 * (cc + 1)], tident)
                        nc.vector.tensor_copy(tYT[:, cc, 128 * tc2:128 * (tc2 + 1)], pt)
                for tc2 in range(8):
                    pc = PSC.tile([128, 1024], f32, name="pc")
                    for cc in range(7):
                        nc.tensor.matmul(pc[:, 0:512], tYT[:, cc, 128 * tc2:128 * (tc2 + 1)],
                                         twp[:, cc, 0:512], start=(cc == 0), stop=(cc == 6))
                        nc.tensor.matmul(pc[:, 512:1024], tYT[:, cc, 128 * tc2:128 * (tc2 + 1)],
                                         twp[:, cc, 512:1024], start=(cc == 0), stop=(cc == 6))
                    ostg = S3.tile([128, 1024], f32, name="ostg")
                    nc.scalar.copy(ostg, pc)
                    nc.sync.dma_start(out=out[128 * tc2:128 * (tc2 + 1), :], in_=ostg)

    nc.finalize()
    return nc


def _get_runner():
    if "runner" in _BUILT:
        return _BUILT["runner"]
    import jax
    import jax.numpy as jnp
    import concourse.mybir as mybir
    from concourse.bass2jax import _bass_exec_p, install_neuronx_cc_hook, partition_id_tensor
    from jax.sharding import Mesh, PartitionSpec, NamedSharding
    from jax.experimental.shard_map import shard_map

    nc = _build_bass()
    install_neuronx_cc_hook()

    # The neuron NEFF cache keys on the HLO module hash, which does NOT cover
    # the embedded BIR content -- a changed bass program would silently reuse a
    # stale NEFF. Salt the cache with a BIR content hash: wipe on mismatch.
    import hashlib, os, shutil
    bir_hash = hashlib.sha256(open(__file__, "rb").read()).hexdigest()[:16]
    cache_root = os.path.expanduser("~/.neuron-compile-cache")
    salt_file = cache_root + "-salt"
    try:
        prev = open(salt_file).read().strip() if os.path.exists(salt_file) else ""
        if prev != bir_hash:
            shutil.rmtree(cache_root, ignore_errors=True)
            os.makedirs(os.path.dirname(salt_file) or "/", exist_ok=True)
            with open(salt_file, "w") as f:
                f.write(bir_hash)
    except OSError:
        pass

    partition_name = nc.partition_id_tensor.name if nc.partition_id_tensor else None
    in_names, out_names, out_avals, zero_shapes = [], [], [], []
    for alloc in nc.m.functions[0].allocations:
        if not isinstance(alloc, mybir.MemoryLocationSet):
            continue
        name = alloc.memorylocations[0].name
        if alloc.kind == "ExternalInput":
            if name != partition_name:
                in_names.append(name)
        elif alloc.kind == "ExternalOutput":
            out_names.append(name)
            shape = tuple(alloc.tensor_shape)
            dtype = mybir.dt.np(alloc.dtype)
            out_avals.append(jax.core.ShapedArray(shape, dtype))
            zero_shapes.append((shape, dtype))
    n_params = len(in_names)
    n_outs = len(out_avals)
    all_in_names = in_names + out_names + ([partition_name] if partition_name else [])
    donate = tuple(range(n_params, n_params + n_outs))

    def _body(*args):
        operands = list(args)
        if partition_name is not None:
            operands.append(partition_id_tensor())
        return tuple(_bass_exec_p.bind(
            *operands, out_avals=tuple(out_avals), in_names=tuple(all_in_names),
            out_names=tuple(out_names), lowering_input_output_aliases=(),
            sim_require_finite=True, sim_require_nnan=True, nc=nc))

    devices = jax.devices()[:N_CORES]
    mesh = Mesh(np.asarray(devices), ("core",))
    sh = NamedSharding(mesh, PartitionSpec("core"))
    sharded = jax.jit(
        shard_map(_body, mesh=mesh,
                  in_specs=(PartitionSpec("core"),) * (n_params + n_outs),
                  out_specs=(PartitionSpec("core"),) * n_outs, check_rep=False),
        donate_argnums=donate, keep_unused=True)

    # zeros made on-device (no H2D per call)
    zmaker = jax.jit(
        lambda: tuple(jnp.zeros((N_CORES * s[0], *s[1:]), dt) for s, dt in zero_shapes),
        out_shardings=(sh,) * n_outs)

    dev_cache = {}

    def run(in_maps, reps=1):
        import time as _time
        concat_dev = []
        for nm in in_names:
            arrs = [np.ascontiguousarray(m[nm]) for m in in_maps]
            key = tuple(hash(a.tobytes()[:4096]) ^ hash(a.tobytes()[-4096:]) ^ a.size
                        for a in arrs)
            hit = dev_cache.get(nm)
            if hit is None or hit[0] != key:
                cat = np.concatenate(arrs, axis=0)
                dev_cache[nm] = (key, jax.device_put(cat, sh))
            concat_dev.append(dev_cache[nm][1])
        jax.block_until_ready(concat_dev)
        t0 = _time.time()
        outs = None
        for _ in range(reps):
            zs = zmaker()
            outs = sharded(*concat_dev, *zs)
        jax.block_until_ready(outs)
        run.last_exec_ns = int((_time.time() - t0) * 1e9 / reps)
        return [
            {name: np.asarray(outs[i]).reshape(N_CORES, *zero_shapes[i][0])[c]
             for i, name in enumerate(out_names)}
            for c in range(N_CORES)
        ]
    run.last_exec_ns = None

    _BUILT["runner"] = run
    return run


def _host_pack(x, alpha_embed, alpha_heads, W_attn, W_proj):
    bf = ml_dtypes.bfloat16
    x = np.asarray(x, np.float32)
    W_attn = np.asarray(W_attn, np.float32)
    W_proj = np.asarray(W_proj, np.float32)
    ae, w = _combo_weights(alpha_embed, alpha_heads)
    s = np.zeros(C_MAX, np.float32)
    for idx, e in enumerate((256, 512, 1024)):
        s[:e] += np.float32(ae[idx])
    row = np.arange(3 * C_MAX) % C_MAX
    col = np.arange(C_MAX)
    Wmix_attn = W_attn * s[np.maximum(row[:, None], col[None, :])]
    Wmix_proj = W_proj * s[np.maximum(col[:, None], col[None, :])]

    tri = np.triu(np.ones((128, 128), np.float32)).astype(bf)  # k <= q in [k,q] layout

    per_parity = {}
    for par in range(2):
        if par == 0:
            feats = list(range(0, 128)) + list(range(0, 256)) + list(range(0, 512))
        else:
            feats = list(range(128, 256)) + list(range(256, 512)) + list(range(512, 1024))
        feats = np.asarray(feats)
        wq = Wmix_attn[0:C_MAX][feats, :]          # [896, 1024]
        wk = Wmix_attn[C_MAX:2 * C_MAX][feats, :]
        wv_ = Wmix_attn[2 * C_MAX:3 * C_MAX][feats, :]
        wqk_flat = np.concatenate([wq.T, wk.T], axis=1).astype(bf)  # [1024, 1792]
        # [p, oc, cc, o]: per-oc DMA reads contiguous [8, 128] per partition
        wqk = np.ascontiguousarray(
            wqk_flat.reshape(8, 128, 14, 128).transpose(1, 2, 0, 3))
        wv = np.ascontiguousarray(wv_.T.astype(bf).reshape(8, 128, PACK_W).transpose(1, 0, 2))
        wp = np.ascontiguousarray(
            Wmix_proj.T[feats, :].astype(bf).reshape(7, 128, C_MAX).transpose(1, 0, 2))
        cf = np.zeros(len(SLICES), np.float32)
        for si, (d, lj) in enumerate(SLICES):
            gj = lj + par * N_LOCAL[d]
            cf[si] = _stair_coef(d, gj, w)
        coefs = np.broadcast_to(cf, (128, len(SLICES))).copy()
        per_parity[par] = dict(wqk=wqk, wv=wv, wp=wp, coefs=coefs)

    in_maps = []
    for core in range(N_CORES):
        b, par = core // 2, core % 2
        pp = per_parity[par]
        in_maps.append({
            "xT": np.ascontiguousarray(x[b].T.astype(bf).reshape(8, 128, T).transpose(1, 0, 2)),
            "wqk": pp["wqk"], "wv": pp["wv"], "wp": pp["wp"],
            "coefs": pp["coefs"], "tri": tri,
        })
    return in_maps


LAST_DEVICE_NS = None


def kernel(x, i=0, alpha_embed=None, alpha_heads=None, W_attn=None, W_proj=None):
    global LAST_DEVICE_NS
    import time
    run = _get_runner()
    in_maps = _host_pack(x, alpha_embed, alpha_heads, W_attn, W_proj)
    res = run(in_maps)
    LAST_DEVICE_NS = run.last_exec_ns
    out = np.empty((B, T, C_MAX), np.float32)
    for b in range(B):
        out[b] = res[2 * b]["out"] + res[2 * b + 1]["out"]
    return out


# revision 14
# speedup vs baseline: 1.5248x; 1.5248x over previous
"""Mixture causal self-attention (NAS weight-entanglement supernet) on 8 trn2 cores.

Math (validated vs reference):
  Wmix = W * s[max(row%C, col)] with staircase s from softmax(alpha_embed).
  qkv = x @ Wmix_attn.T ; y = sum over 9 (h,e) combos of w_he * Attn_he(q,k,v);
  out = y @ Wmix_proj.T.
Key reduction: combos with equal head dim d=e/h produce IDENTICAL per-slice
attention outputs, so the 9 combos collapse to 60 unique (d, slice) units,
accumulated with staircase weights:
  d=16: 16 slices, d=32: 16, d=64: 16, d=128: 8, d=256: 4.
Sharding: core pair (2b, 2b+1) owns batch b; even cores take the first half of
each d-group's slices (features [0:512]), odd cores the second half. Identical
SPMD program; per-core weight/coef data differ. Host sums the pair partials.
"""

import numpy as np
import ml_dtypes

C_MAX = 1024
T = 1024
B = 4
N_CORES = 8

# local slice list (per core), in processing order: copies into y before adds
# (d64 tiles y cols [384:896] disjointly -> copy; d128/d256 overlap -> add)
SLICES = (
    [(64, j) for j in range(8)]
    + [(128, j) for j in range(4)]
    + [(256, j) for j in range(2)]
    + [(32, j) for j in range(8)]
    + [(16, j) for j in range(8)]
)
N_LOCAL = {16: 8, 32: 8, 64: 8, 128: 4, 256: 2}
# packed q/k row space [0:896]: small = d16 feats, mid = d32 feats, big = rest
QOFF = {16: 0, 32: 128}  # d64/128/256 live in big block [384:896] via tQ128
# Vx (V with interleaved ones col per slice) group offsets
VXOFF = {}
_off = 0
for _d in (64, 128, 256, 32, 16):
    VXOFF[_d] = _off
    _off += (_d + 1) * N_LOCAL[_d]
VX_W = _off  # 1950
# y packed col offsets (no ones cols)
YOFF = {16: 0, 32: 128, 64: 384, 128: 384, 256: 384}
PACK_W = 896

_BUILT = {}


def _softmax1d(v):
    v = v - v.max()
    e = np.exp(v)
    return e / e.sum()


def _combo_weights(alpha_embed, alpha_heads):
    ae = _softmax1d(np.asarray(alpha_embed, np.float64))
    ah = _softmax1d(np.asarray(alpha_heads, np.float64))
    w = {}
    for hi, h in enumerate((4, 8, 16)):
        for ei, e in enumerate((256, 512, 1024)):
            w[(h, e)] = float(ah[hi] * ae[ei])
    return ae, w


def _stair_coef(d, gj, w):
    # weight of global slice (d, gj) = sum of w[(h, h*d)] over combos with h > gj
    c = 0.0
    for h in (4, 8, 16):
        e = h * d
        if e in (256, 512, 1024) and gj < h:
            c += w[(h, e)]
    return c


def _build_bass():
    import concourse.bass as bass
    from concourse import bacc
    import concourse.mybir as mybir
    import concourse.tile as tile
    from concourse.masks import make_identity

    bf16 = mybir.dt.bfloat16
    f32 = mybir.dt.float32
    AF = mybir.ActivationFunctionType

    nc = bacc.Bacc()
    xT = nc.dram_tensor("xT", [128, 8, T], bf16, kind="ExternalInput")
    wqk = nc.dram_tensor("wqk", [128, 14, 8, 128], bf16, kind="ExternalInput")
    wv = nc.dram_tensor("wv", [128, 8, PACK_W], bf16, kind="ExternalInput")
    wp = nc.dram_tensor("wp", [128, 7, C_MAX], bf16, kind="ExternalInput")
    coefs = nc.dram_tensor("coefs", [128, len(SLICES)], f32, kind="ExternalInput")
    tri = nc.dram_tensor("tri", [128, 128], bf16, kind="ExternalInput")
    out = nc.dram_tensor("out", [T, C_MAX], f32, kind="ExternalOutput")

    # E strips packed into 5 groups: (kb0), (kb1,kb7), (kb2,kb6), (kb3,kb5), (kb4)
    # one PSUM tile + one exp per group
    KB_GROUPS = [(0,), (1, 7), (2, 6), (3, 5), (4,)]
    EOFF = {}
    _e = 0
    for grp in KB_GROUPS:
        for kb in grp:
            EOFF[kb] = _e
            _e += 1024 - 128 * kb
    E_W = _e  # 4608
    # within-psum-tile col offset of each kb (strips packed in group order)
    PSOFF = {}
    for grp in KB_GROUPS:
        _o = 0
        for kb in grp:
            PSOFF[kb] = _o
            _o += 1024 - 128 * kb

    with tile.TileContext(nc) as tc:
        with tc.tile_pool(name="persist", bufs=1) as P:
            ttri = P.tile([128, 128], bf16, name="ttri")
            nc.sync.dma_start(out=ttri, in_=tri[:, :])
            tcoef = P.tile([128, len(SLICES)], f32, name="tcoef")
            nc.sync.dma_start(out=tcoef, in_=coefs[:, :])
            tident = P.tile([128, 128], f32, name="tident")
            make_identity(nc, tident)

            # persistent activation tensors
            tQ16 = P.tile([16, 8, 1024], bf16, name="tQ16")
            tK16 = P.tile([16, 8, 1024], bf16, name="tK16")
            tQ32 = P.tile([32, 8, 1024], bf16, name="tQ32")
            tK32 = P.tile([32, 8, 1024], bf16, name="tK32")
            tQ128 = P.tile([128, 4, 1024], bf16, name="tQ128")
            tK128 = P.tile([128, 4, 1024], bf16, name="tK128")
            tVx = P.tile([128, 8, VX_W], bf16, name="tVx")
            tY = P.tile([128, 8, PACK_W], f32, name="tY")

            GQ = {16: tQ16, 32: tQ32}
            GK = {16: tK16, 32: tK32}

            # ---------------- phase 1: qkv projection ----------------
            with tc.tile_pool(name="p1sb", bufs=3) as S1, \
                 tc.tile_pool(name="p1w", bufs=4) as WP, \
                 tc.tile_pool(name="p1x", bufs=1) as XP, \
                 tc.tile_pool(name="pqk", bufs=2, space="PSUM") as PQK, \
                 tc.tile_pool(name="pv", bufs=2, space="PSUM") as PV:
                tX = XP.tile([128, 8, 1024], bf16, name="tX")
                nc.sync.dma_start(out=tX, in_=xT[:, :, :])
                twv = XP.tile([128, 8, PACK_W], bf16, name="twv")
                nc.sync.dma_start(out=twv, in_=wv[:, :, :])

                # Q^T / K^T packed rows, o-chunks of 128
                for oc in range(14):
                    pqk = PQK.tile([128, 1024], f32, name="pqk")
                    wt = WP.tile([128, 8, 128], bf16, name="wt")
                    nc.sync.dma_start(out=wt, in_=wqk[:, oc, :, :])
                    for cc in range(8):
                        nc.tensor.matmul(pqk[:, 0:512], wt[:, cc, :], tX[:, cc, 0:512],
                                         start=(cc == 0), stop=(cc == 7))
                        nc.tensor.matmul(pqk[:, 512:1024], wt[:, cc, :], tX[:, cc, 512:1024],
                                         start=(cc == 0), stop=(cc == 7))
                    isq, sub = divmod(oc, 7)
                    gdst = (GQ if isq == 0 else GK)
                    if sub == 0:  # small block: 8 d16 slices of 16 rows
                        stg = S1.tile([128, 1024], bf16, name="stg")
                        nc.vector.tensor_copy(stg, pqk)
                        for s in range(8):
                            nc.sync.dma_start(out=gdst[16][:, s, :], in_=stg[16 * s:16 * (s + 1), :])
                    elif sub in (1, 2):  # mid block: 4 d32 slices per chunk
                        stg = S1.tile([128, 1024], bf16, name="stg")
                        nc.vector.tensor_copy(stg, pqk)
                        for s in range(4):
                            nc.sync.dma_start(out=gdst[32][:, 4 * (sub - 1) + s, :],
                                              in_=stg[32 * s:32 * (s + 1), :])
                    else:  # big block: direct
                        dst = (tQ128 if isq == 0 else tK128)
                        nc.vector.tensor_copy(dst[:, sub - 3, :], pqk)

                # V (normal layout) -> interleaved Vx with ones columns
                for tc2 in range(8):
                    pv = PV.tile([128, 896], f32, name="pv")
                    for cc in range(8):
                        nc.tensor.matmul(pv[:, 0:512], tX[:, cc, 128 * tc2:128 * (tc2 + 1)],
                                         twv[:, cc, 0:512], start=(cc == 0), stop=(cc == 7))
                        nc.tensor.matmul(pv[:, 512:896], tX[:, cc, 128 * tc2:128 * (tc2 + 1)],
                                         twv[:, cc, 512:896], start=(cc == 0), stop=(cc == 7))
                    for d in (64, 128, 256, 32, 16):
                        n = N_LOCAL[d]
                        # copy [128, n, d] from packed V cols -> Vx strided (d+1)
                        voff = YOFF[d]
                        nc.vector.tensor_copy(
                            tVx[:, tc2, VXOFF[d]:VXOFF[d] + (d + 1) * n]
                            .rearrange("p (s e) -> p s e", e=d + 1)[:, :, 0:d],
                            pv[:, voff:voff + d * n].rearrange("p (s e) -> p s e", e=d))
                # ones columns
                for d in (64, 128, 256, 32, 16):
                    n = N_LOCAL[d]
                    nc.vector.memset(
                        tVx[:, :, VXOFF[d]:VXOFF[d] + (d + 1) * n]
                        .rearrange("p t (s e) -> p t s e", e=d + 1)[:, :, :, d:d + 1], 1.0)

            # ---------------- phase 2: attention slices ----------------
            # software pipeline: scores+exp for slice pair p overlap E@V of
            # pair p-1 (PE works E@V while ACT runs exp of the next pair)
            with tc.tile_pool(name="p2e", bufs=5) as EP, \
                 tc.tile_pool(name="p2v", bufs=6) as DVP, \
                 tc.tile_pool(name="pss", bufs=3, space="PSUM") as PSS, \
                 tc.tile_pool(name="pso", bufs=2, space="PSUM") as PSO:

                def emit_scores_group(d, lj, tE, grp, scale):
                    ps = PSS.tile([128, 1024], f32, name="ps")
                    gw = 0
                    for kb in grp:
                        w = 1024 - 128 * kb
                        base = PSOFF[kb]
                        # matmul out chunks within one psum bank each
                        cuts = [base]
                        for b in (512, 1024):
                            if base < b < base + w:
                                cuts.append(b)
                        cuts.append(base + w)
                        if d <= 64:
                            if d <= 32:
                                lk = GK[d][:, lj, :]
                                lq = GQ[d][:, lj, :]
                            else:
                                p0 = 64 * (lj % 2)
                                lk = tK128[p0:p0 + 64, lj // 2, :]
                                lq = tQ128[p0:p0 + 64, lj // 2, :]
                            for a, b in zip(cuts[:-1], cuts[1:]):
                                qo = 128 * kb + (a - base)
                                nc.tensor.matmul(
                                    ps[:, a:b], lk[:, 128 * kb:128 * (kb + 1)],
                                    lq[:, qo:qo + (b - a)], start=True, stop=True)
                        elif d == 128:
                            for a, b in zip(cuts[:-1], cuts[1:]):
                                qo = 128 * kb + (a - base)
                                nc.tensor.matmul(
                                    ps[:, a:b], tK128[:, lj, 128 * kb:128 * (kb + 1)],
                                    tQ128[:, lj, qo:qo + (b - a)], start=True, stop=True)
                        else:
                            for a, b in zip(cuts[:-1], cuts[1:]):
                                qo = 128 * kb + (a - base)
                                for h2 in range(2):
                                    nc.tensor.matmul(
                                        ps[:, a:b], tK128[:, 2 * lj + h2, 128 * kb:128 * (kb + 1)],
                                        tQ128[:, 2 * lj + h2, qo:qo + (b - a)],
                                        start=(h2 == 0), stop=(h2 == 1))
                        gw += w
                    # one exp for the whole packed group
                    kb0 = grp[0]
                    nc.scalar.activation(tE[:, EOFF[kb0]:EOFF[kb0] + gw], ps[:, 0:gw],
                                         AF.Exp, scale=scale)
                    for kb in grp:
                        nc.gpsimd.affine_select(
                            out=tE[:, EOFF[kb]:EOFF[kb] + 128],
                            in_=tE[:, EOFF[kb]:EOFF[kb] + 128],
                            compare_op=mybir.AluOpType.is_ge,
                            fill=0.0, base=0, pattern=[[1, 128]], channel_multiplier=-1)

                def emit_eav(si, d, lj, tE):
                    for qi in range(8):
                        po = PSO.tile([128, 257], f32, name="po")
                        for kb in range(qi + 1):
                            nc.tensor.matmul(
                                po[:, 0:d + 1],
                                tE[:, EOFF[kb] + 128 * (qi - kb):EOFF[kb] + 128 * (qi - kb) + 128],
                                tVx[:, kb, VXOFF[d] + (d + 1) * lj:VXOFF[d] + (d + 1) * (lj + 1)],
                                start=(kb == 0), stop=(kb == qi))
                        tdin = DVP.tile([128, 1], f32, name="tdin")
                        nc.vector.reciprocal(tdin, po[:, d:d + 1])
                        ycol = YOFF[d] + d * lj
                        if d in (128, 256):  # accumulate
                            ttmp = DVP.tile([128, 256], f32, name="ttmp")
                            nc.vector.tensor_scalar(
                                out=ttmp[:, 0:d], in0=po[:, 0:d], scalar1=tdin,
                                scalar2=tcoef[:, si:si + 1],
                                op0=mybir.AluOpType.mult, op1=mybir.AluOpType.mult)
                            nc.vector.tensor_add(tY[:, qi, ycol:ycol + d],
                                                 tY[:, qi, ycol:ycol + d], ttmp[:, 0:d])
                        else:  # first (only) writer of these y cols
                            nc.vector.tensor_scalar(
                                out=tY[:, qi, ycol:ycol + d], in0=po[:, 0:d], scalar1=tdin,
                                scalar2=tcoef[:, si:si + 1],
                                op0=mybir.AluOpType.mult, op1=mybir.AluOpType.mult)

                prev = None
                for pi in range(0, len(SLICES), 2):
                    cur = []
                    for si in (pi, pi + 1):
                        d, lj = SLICES[si]
                        tE = EP.tile([128, E_W], bf16, name="tE")
                        cur.append((si, d, lj, tE))
                    for grp in KB_GROUPS:
                        for (si, d, lj, tE) in cur:
                            emit_scores_group(d, lj, tE, grp, float(1.0 / np.sqrt(d)))
                    if prev is not None:
                        for (si, d, lj, tE) in prev:
                            emit_eav(si, d, lj, tE)
                    prev = cur
                for (si, d, lj, tE) in prev:
                    emit_eav(si, d, lj, tE)

            # ---------------- phase 3: transpose y, c_proj ----------------
            with tc.tile_pool(name="p3sb", bufs=2) as S3, \
                 tc.tile_pool(name="p3yt", bufs=1) as YTP, \
                 tc.tile_pool(name="p3w", bufs=1) as WP3, \
                 tc.tile_pool(name="pst", bufs=2, space="PSUM") as PST, \
                 tc.tile_pool(name="psc", bufs=2, space="PSUM") as PSC:
                twp = WP3.tile([128, 7, 1024], bf16, name="twp")
                nc.sync.dma_start(out=twp, in_=wp[:, :, :])
                tYT = YTP.tile([128, 7, 1024], bf16, name="tYT")
                for cc in range(7):
                    for tc2 in range(8):
                        pt = PST.tile([128, 128], f32, name="pt")
                        nc.tensor.transpose(pt, tY[:, tc2, 128 * cc:128 * (cc + 1)], tident)
                        nc.vector.tensor_copy(tYT[:, cc, 128 * tc2:128 * (tc2 + 1)], pt)
                for tc2 in range(8):
                    pc = PSC.tile([128, 1024], f32, name="pc")
                    for cc in range(7):
                        nc.tensor.matmul(pc[:, 0:512], tYT[:, cc, 128 * tc2:128 * (tc2 + 1)],
                                         twp[:, cc, 0:512], start=(cc == 0), stop=(cc == 6))
                        nc.tensor.matmul(pc[:, 512:1024], tYT[:, cc, 128 * tc2:128 * (tc2 + 1)],
                                         twp[:, cc, 512:1024], start=(cc == 0), stop=(cc == 6))
                    ostg = S3.tile([128, 1024], f32, name="ostg")
                    nc.scalar.copy(ostg, pc)
                    nc.sync.dma_start(out=out[128 * tc2:128 * (tc2 + 1), :], in_=ostg)

    nc.finalize()
    return nc


def _get_runner():
    if "runner" in _BUILT:
        return _BUILT["runner"]
    import jax
    import jax.numpy as jnp
    import concourse.mybir as mybir
    from concourse.bass2jax import _bass_exec_p, install_neuronx_cc_hook, partition_id_tensor
    from jax.sharding import Mesh, PartitionSpec, NamedSharding
    from jax.experimental.shard_map import shard_map

    nc = _build_bass()
    install_neuronx_cc_hook()

    # The neuron NEFF cache keys on the HLO module hash, which does NOT cover
    # the embedded BIR content -- a changed bass program would silently reuse a
    # stale NEFF. Salt the cache with a BIR content hash: wipe on mismatch.
    import hashlib, os, shutil
    bir_hash = hashlib.sha256(open(__file__, "rb").read()).hexdigest()[:16]
    cache_root = os.path.expanduser("~/.neuron-compile-cache")
    salt_file = cache_root + "-salt"
    try:
        prev = open(salt_file).read().strip() if os.path.exists(salt_file) else ""
        if prev != bir_hash:
            shutil.rmtree(cache_root, ignore_errors=True)
            os.makedirs(os.path.dirname(salt_file) or "/", exist_ok=True)
            with open(salt_file, "w") as f:
                f.write(bir_hash)
    except OSError:
        pass

    partition_name = nc.partition_id_tensor.name if nc.partition_id_tensor else None
    in_names, out_names, out_avals, zero_shapes = [], [], [], []
    for alloc in nc.m.functions[0].allocations:
        if not isinstance(alloc, mybir.MemoryLocationSet):
            continue
        name = alloc.memorylocations[0].name
        if alloc.kind == "ExternalInput":
            if name != partition_name:
                in_names.append(name)
        elif alloc.kind == "ExternalOutput":
            out_names.append(name)
            shape = tuple(alloc.tensor_shape)
            dtype = mybir.dt.np(alloc.dtype)
            out_avals.append(jax.core.ShapedArray(shape, dtype))
            zero_shapes.append((shape, dtype))
    n_params = len(in_names)
    n_outs = len(out_avals)
    all_in_names = in_names + out_names + ([partition_name] if partition_name else [])
    donate = tuple(range(n_params, n_params + n_outs))

    def _body(*args):
        operands = list(args)
        if partition_name is not None:
            operands.append(partition_id_tensor())
        return tuple(_bass_exec_p.bind(
            *operands, out_avals=tuple(out_avals), in_names=tuple(all_in_names),
            out_names=tuple(out_names), lowering_input_output_aliases=(),
            sim_require_finite=True, sim_require_nnan=True, nc=nc))

    devices = jax.devices()[:N_CORES]
    mesh = Mesh(np.asarray(devices), ("core",))
    sh = NamedSharding(mesh, PartitionSpec("core"))
    sharded = jax.jit(
        shard_map(_body, mesh=mesh,
                  in_specs=(PartitionSpec("core"),) * (n_params + n_outs),
                  out_specs=(PartitionSpec("core"),) * n_outs, check_rep=False),
        donate_argnums=donate, keep_unused=True)

    # zeros made on-device (no H2D per call)
    zmaker = jax.jit(
        lambda: tuple(jnp.zeros((N_CORES * s[0], *s[1:]), dt) for s, dt in zero_shapes),
        out_shardings=(sh,) * n_outs)

    dev_cache = {}

    def run(in_maps, reps=1):
        import time as _time
        concat_dev = []
        for nm in in_names:
            arrs = [np.ascontiguousarray(m[nm]) for m in in_maps]
            key = tuple(hash(a.tobytes()[:4096]) ^ hash(a.tobytes()[-4096:]) ^ a.size
                        for a in arrs)
            hit = dev_cache.get(nm)
            if hit is None or hit[0] != key:
                cat = np.concatenate(arrs, axis=0)
                dev_cache[nm] = (key, jax.device_put(cat, sh))
            concat_dev.append(dev_cache[nm][1])
        jax.block_until_ready(concat_dev)
        t0 = _time.time()
        outs = None
        for _ in range(reps):
            zs = zmaker()
            outs = sharded(*concat_dev, *zs)
        jax.block_until_ready(outs)
        run.last_exec_ns = int((_time.time() - t0) * 1e9 / reps)
        return [
            {name: np.asarray(outs[i]).reshape(N_CORES, *zero_shapes[i][0])[c]
             for i, name in enumerate(out_names)}
            for c in range(N_CORES)
        ]
    run.last_exec_ns = None

    _BUILT["runner"] = run
    return run


def _host_pack(x, alpha_embed, alpha_heads, W_attn, W_proj):
    bf = ml_dtypes.bfloat16
    x = np.asarray(x, np.float32)
    W_attn = np.asarray(W_attn, np.float32)
    W_proj = np.asarray(W_proj, np.float32)
    ae, w = _combo_weights(alpha_embed, alpha_heads)
    s = np.zeros(C_MAX, np.float32)
    for idx, e in enumerate((256, 512, 1024)):
        s[:e] += np.float32(ae[idx])
    row = np.arange(3 * C_MAX) % C_MAX
    col = np.arange(C_MAX)
    Wmix_attn = W_attn * s[np.maximum(row[:, None], col[None, :])]
    Wmix_proj = W_proj * s[np.maximum(col[:, None], col[None, :])]

    tri = np.triu(np.ones((128, 128), np.float32)).astype(bf)  # k <= q in [k,q] layout

    per_parity = {}
    for par in range(2):
        if par == 0:
            feats = list(range(0, 128)) + list(range(0, 256)) + list(range(0, 512))
        else:
            feats = list(range(128, 256)) + list(range(256, 512)) + list(range(512, 1024))
        feats = np.asarray(feats)
        wq = Wmix_attn[0:C_MAX][feats, :]          # [896, 1024]
        wk = Wmix_attn[C_MAX:2 * C_MAX][feats, :]
        wv_ = Wmix_attn[2 * C_MAX:3 * C_MAX][feats, :]
        wqk_flat = np.concatenate([wq.T, wk.T], axis=1).astype(bf)  # [1024, 1792]
        # [p, oc, cc, o]: per-oc DMA reads contiguous [8, 128] per partition
        wqk = np.ascontiguousarray(
            wqk_flat.reshape(8, 128, 14, 128).transpose(1, 2, 0, 3))
        wv = np.ascontiguousarray(wv_.T.astype(bf).reshape(8, 128, PACK_W).transpose(1, 0, 2))
        wp = np.ascontiguousarray(
            Wmix_proj.T[feats, :].astype(bf).reshape(7, 128, C_MAX).transpose(1, 0, 2))
        cf = np.zeros(len(SLICES), np.float32)
        for si, (d, lj) in enumerate(SLICES):
            gj = lj + par * N_LOCAL[d]
            cf[si] = _stair_coef(d, gj, w)
        coefs = np.broadcast_to(cf, (128, len(SLICES))).copy()
        per_parity[par] = dict(wqk=wqk, wv=wv, wp=wp, coefs=coefs)

    in_maps = []
    for core in range(N_CORES):
        b, par = core // 2, core % 2
        pp = per_parity[par]
        in_maps.append({
            "xT": np.ascontiguousarray(x[b].T.astype(bf).reshape(8, 128, T).transpose(1, 0, 2)),
            "wqk": pp["wqk"], "wv": pp["wv"], "wp": pp["wp"],
            "coefs": pp["coefs"], "tri": tri,
        })
    return in_maps


LAST_DEVICE_NS = None


def kernel(x, i=0, alpha_embed=None, alpha_heads=None, W_attn=None, W_proj=None):
    global LAST_DEVICE_NS
    import time
    run = _get_runner()
    in_maps = _host_pack(x, alpha_embed, alpha_heads, W_attn, W_proj)
    res = run(in_maps)
    LAST_DEVICE_NS = run.last_exec_ns
    out = np.empty((B, T, C_MAX), np.float32)
    for b in range(B):
        out[b] = res[2 * b]["out"] + res[2 * b + 1]["out"]
    return out
